# revision 11
# baseline (speedup 1.0000x reference)
"""CAM (channel attention module) kernel for trn2, batch-parallel over 8 cores.

Per batch item b (x_b: [C=512, N=4096] fp32):
  E = x_b @ x_b.T                  [C, C]   (fp32r matmuls, contraction over N)
  att = softmax(rowmax(E) - E)              == row-softmax of (-E) (shift invariant)
  out = gamma * (att @ x_b) + x_b           (residual folded in as (gamma*att + I) @ x)

Each core handles B/8 = 4 batch items. Inputs arrive full; sharded here.
"""

import numpy as np

B, C, N = 32, 512, 4096
N_CORES = 8
B_PER = B // N_CORES  # 4
P = 128
CT = C // P   # 4 c-tiles
KT = N // P   # 32 k-tiles over N
NF = 512      # matmul2 free-dim chunk (one PSUM bank of fp32)
NCH = N // NF  # 8

_CACHE = {}


def _build_bass():
    import concourse.bass as bass
    import concourse.mybir as mybir
    import concourse.tile as tile
    from concourse import masks

    f32 = mybir.dt.float32
    f32r = mybir.dt.float32r
    AX = mybir.AxisListType
    OP = mybir.AluOpType
    AF = mybir.ActivationFunctionType

    nc = bass.Bass("TRN2", debug=False)
    x_d = nc.dram_tensor("x", [B_PER, C, N], f32, kind="ExternalInput")
    g_d = nc.dram_tensor("gamma", [1], f32, kind="ExternalInput")
    o_d = nc.dram_tensor("out", [B_PER, C, N], f32, kind="ExternalOutput")
    x_ap, g_ap, o_ap = x_d.ap(), g_d.ap(), o_d.ap()

    def copy_on(eng, out, in_):
        if eng is nc.scalar:
            nc.scalar.copy(out=out, in_=in_)
        else:
            nc.vector.tensor_copy(out=out, in_=in_)

    with tile.TileContext(nc) as tc:
        with (
            tc.tile_pool(name="xnat", bufs=2) as xnat_pool,
            tc.tile_pool(name="xt", bufs=4) as xt_pool,
            tc.tile_pool(name="consts", bufs=1) as const_pool,
            tc.tile_pool(name="expt", bufs=2) as expt_pool,
            tc.tile_pool(name="attm", bufs=6) as attm_pool,
            tc.tile_pool(name="attT", bufs=6) as attT_pool,
            tc.tile_pool(name="stats", bufs=16) as stat_pool,
            tc.tile_pool(name="outsb", bufs=2) as out_pool,
            tc.tile_pool(name="ps", bufs=4, space="PSUM") as ps_pool,
            tc.tile_pool(name="eps", bufs=4, space="PSUM") as e_pool,
        ):
            ident = const_pool.tile([P, P], f32)
            masks.make_identity(nc, ident[:])
            ident_r = const_pool.tile([P, P], f32r)
            nc.vector.tensor_copy(out=ident_r[:], in_=ident[:])
            gamma_sb = const_pool.tile([P, 1], f32)
            nc.gpsimd.dma_start(out=gamma_sb[:], in_=g_ap.to_broadcast((P, 1)))

            for b in range(B_PER):
                # ---- load x_b as 4 row-tiles [128, 4096] (one 8MB DMA) ----
                xnat = xnat_pool.tile([P, CT, N], f32r)
                nc.sync.dma_start(
                    out=xnat[:],
                    in_=x_ap[b].rearrange("(j p) n -> p j n", p=P).bitcast(f32r),
                )

                # ---- phase 1: transpose x + E = x @ x.T (accumulate over kn) ----
                e_tiles = [e_pool.tile([P, C], f32, tag="e", name="e_t") for _ in range(CT)]
                for kn in range(KT):
                    xt = xt_pool.tile([P, C], f32r)
                    for j in range(CT):
                        tp = ps_pool.tile([P, P], f32r, tag="ps", name="tp")
                        nc.tensor.transpose(
                            tp[:], xnat[:, j, kn * P : (kn + 1) * P], ident_r[:]
                        )
                        eng = nc.scalar if kn % 2 == 0 else nc.vector
                        copy_on(eng, xt[:, j * P : (j + 1) * P], tp[:])
                    for ct in range(CT):
                        nc.tensor.matmul(
                            e_tiles[ct][:],
                            lhsT=xt[:, ct * P : (ct + 1) * P],
                            rhs=xt[:],
                            start=(kn == 0),
                            stop=(kn == KT - 1),
                        )

                # ---- phase 2: row softmax of (-E), scaled by gamma ----
                attm_tiles = []
                for ct in range(CT):
                    m = stat_pool.tile([P, 1], f32, tag="m")
                    nc.vector.tensor_reduce(
                        m[:], e_tiles[ct][:], axis=AX.X, op=OP.min
                    )
                    s = stat_pool.tile([P, 1], f32, tag="s")
                    expt = expt_pool.tile([P, C], f32)
                    # exp(m - E), with row sums accumulated into s
                    nc.scalar.activation(
                        expt[:], e_tiles[ct][:], AF.Exp,
                        bias=m[:], scale=-1.0, accum_out=s[:],
                    )
                    r = stat_pool.tile([P, 1], f32, tag="r")
                    nc.vector.reciprocal(r[:], s[:])
                    rg = stat_pool.tile([P, 1], f32, tag="rg")
                    nc.vector.tensor_mul(rg[:], r[:], gamma_sb[:])
                    attm = attm_pool.tile([P, C], f32)
                    nc.vector.tensor_scalar_mul(attm[:], expt[:], rg[:])
                    attm_tiles.append(attm)

                # ---- phase 3: attT = (gamma*att).T + I ----
                attT_tiles = [attT_pool.tile([P, C], f32r, tag="attT", name="attT_t") for _ in range(CT)]
                for ct in range(CT):
                    for dt in range(CT):
                        tp = ps_pool.tile([P, P], f32, tag="ps", name="tp2")
                        nc.tensor.transpose(
                            tp[:], attm_tiles[ct][:, dt * P : (dt + 1) * P], ident[:]
                        )
                        dst = attT_tiles[dt][:, ct * P : (ct + 1) * P]
                        if ct == dt:
                            nc.vector.tensor_add(dst, tp[:], ident[:])
                        else:
                            nc.vector.tensor_copy(out=dst, in_=tp[:])

                # ---- phase 4: out = (gamma*att + I).T.T @ x ----
                for ct in range(CT):
                    osb = out_pool.tile([P, N], f32)
                    for nch in range(NCH):
                        ops = ps_pool.tile([P, NF], f32, tag="ps")
                        for dt in range(CT):
                            nc.tensor.matmul(
                                ops[:],
                                lhsT=attT_tiles[dt][:, ct * P : (ct + 1) * P],
                                rhs=xnat[:, dt, nch * NF : (nch + 1) * NF],
                                start=(dt == 0),
                                stop=(dt == CT - 1),
                            )
                        eng = nc.scalar if ct % 2 == 0 else nc.vector
                        copy_on(eng, osb[:, nch * NF : (nch + 1) * NF], ops[:])
                    nc.sync.dma_start(
                        out=o_ap[b, ct * P : (ct + 1) * P, :], in_=osb[:]
                    )
    return nc


def _split_excess_waits(nc, max_waits=1):
    """Walrus codegen rejects instructions with more sync-waits than the HW
    instruction struct can hold (self-loading fp32 matmuls hold just one).
    Hoist excess waits onto standalone InstEventSemaphore ops (the same thing
    engine.wait_ge emits) placed immediately before, on the same engine."""
    import concourse.mybir as mybir

    ctr = 0
    for fn in nc.m.functions:
        for blk in fn.blocks:
            out = []
            for inst in blk.instructions:
                si = getattr(inst, "sync_info", None)
                if si is not None and si.on_wait and len(si.on_wait) > max_waits:
                    extra, keep = si.on_wait[:-max_waits], si.on_wait[-max_waits:]
                    for w in extra:
                        ctr += 1
                        ev = mybir.InstEventSemaphore(
                            name=f"WSPLIT-{ctr}", ins=[], outs=[]
                        )
                        ev.engine = inst.engine
                        ev.sync_info = mybir.SyncInfo(on_wait=[w], on_update=[])
                        out.append(ev)
                    inst.sync_info = mybir.SyncInfo(
                        on_wait=keep, on_update=si.on_update
                    )
                out.append(inst)
            blk.instructions[:] = out
    return ctr


def _get_nc():
    if "nc" not in _CACHE:
        nc = _build_bass()
        _split_excess_waits(nc)
        _CACHE["nc"] = nc
    return _CACHE["nc"]


def kernel(x: np.ndarray, gamma: np.ndarray) -> np.ndarray:
    from concourse import bass_utils

    x = np.ascontiguousarray(np.asarray(x), dtype=np.float32)
    gamma = np.ascontiguousarray(np.asarray(gamma), dtype=np.float32)
    nc = _get_nc()
    in_maps = [
        {"x": x[i * B_PER : (i + 1) * B_PER], "gamma": gamma} for i in range(N_CORES)
    ]
    res = bass_utils.run_bass_kernel_spmd(nc, in_maps, core_ids=list(range(N_CORES)))
    out = np.concatenate([r["out"] for r in res.results], axis=0)
    return out


# revision 14
# speedup vs baseline: 190.4383x; 190.4383x over previous
"""CAM (channel attention module) kernel for trn2, batch-parallel over 8 cores.

Per batch item b (x_b: [C=512, N=4096] fp32):
  E = x_b @ x_b.T                  [C, C]   (fp32r matmuls, contraction over N)
  att = softmax(rowmax(E) - E)              == row-softmax of (-E) (shift invariant)
  out = gamma * (att @ x_b) + x_b           (residual folded in as (gamma*att + I) @ x)

Each core handles B/8 = 4 batch items. Inputs arrive full; sharded here.
"""

import numpy as np

B, C, N = 32, 512, 4096
N_CORES = 8
B_PER = B // N_CORES  # 4
P = 128
CT = C // P   # 4 c-tiles
KT = N // P   # 32 k-tiles over N
NF = 512      # matmul2 free-dim chunk (one PSUM bank of fp32)
NCH = N // NF  # 8

_CACHE = {}


def _build_bass():
    import concourse.bass as bass
    import concourse.mybir as mybir
    import concourse.tile as tile
    from concourse import masks

    f32 = mybir.dt.float32
    f32r = mybir.dt.float32r
    AX = mybir.AxisListType
    OP = mybir.AluOpType
    AF = mybir.ActivationFunctionType

    nc = bass.Bass("TRN2", debug=False)
    x_d = nc.dram_tensor("x", [B_PER, C, N], f32, kind="ExternalInput")
    g_d = nc.dram_tensor("gamma", [1], f32, kind="ExternalInput")
    o_d = nc.dram_tensor("out", [B_PER, C, N], f32, kind="ExternalOutput")
    x_ap, g_ap, o_ap = x_d.ap(), g_d.ap(), o_d.ap()

    def copy_on(eng, out, in_):
        if eng is nc.scalar:
            nc.scalar.copy(out=out, in_=in_)
        else:
            nc.vector.tensor_copy(out=out, in_=in_)

    with tile.TileContext(nc) as tc:
        with (
            tc.tile_pool(name="xnat", bufs=2) as xnat_pool,
            tc.tile_pool(name="xt", bufs=4) as xt_pool,
            tc.tile_pool(name="consts", bufs=1) as const_pool,
            tc.tile_pool(name="expt", bufs=2) as expt_pool,
            tc.tile_pool(name="attm", bufs=6) as attm_pool,
            tc.tile_pool(name="attT", bufs=6) as attT_pool,
            tc.tile_pool(name="stats", bufs=16) as stat_pool,
            tc.tile_pool(name="outsb", bufs=2) as out_pool,
            tc.tile_pool(name="ps", bufs=4, space="PSUM") as ps_pool,
            tc.tile_pool(name="eps", bufs=4, space="PSUM") as e_pool,
        ):
            ident = const_pool.tile([P, P], f32)
            masks.make_identity(nc, ident[:])
            ident_r = const_pool.tile([P, P], f32r)
            nc.vector.tensor_copy(out=ident_r[:], in_=ident[:])
            gamma_sb = const_pool.tile([P, 1], f32)
            nc.gpsimd.dma_start(out=gamma_sb[:], in_=g_ap.to_broadcast((P, 1)))

            for b in range(B_PER):
                # ---- load x_b as 4 row-tiles [128, 4096] (one 8MB DMA) ----
                xnat = xnat_pool.tile([P, CT, N], f32r)
                nc.sync.dma_start(
                    out=xnat[:],
                    in_=x_ap[b].rearrange("(j p) n -> p j n", p=P).bitcast(f32r),
                )

                # ---- phase 1: transpose x + E = x @ x.T (accumulate over kn) ----
                e_tiles = [e_pool.tile([P, C], f32, tag="e", name="e_t") for _ in range(CT)]
                for kn in range(KT):
                    xt = xt_pool.tile([P, C], f32r)
                    for j in range(CT):
                        tp = ps_pool.tile([P, P], f32r, tag="ps", name="tp")
                        nc.tensor.transpose(
                            tp[:], xnat[:, j, kn * P : (kn + 1) * P], ident_r[:]
                        )
                        eng = nc.scalar if kn % 2 == 0 else nc.vector
                        copy_on(eng, xt[:, j * P : (j + 1) * P], tp[:])
                    for ct in range(CT):
                        nc.tensor.matmul(
                            e_tiles[ct][:],
                            lhsT=xt[:, ct * P : (ct + 1) * P],
                            rhs=xt[:],
                            start=(kn == 0),
                            stop=(kn == KT - 1),
                        )

                # ---- phase 2: row softmax of (-E), scaled by gamma ----
                attm_tiles = []
                for ct in range(CT):
                    m = stat_pool.tile([P, 1], f32, tag="m")
                    nc.vector.tensor_reduce(
                        m[:], e_tiles[ct][:], axis=AX.X, op=OP.min
                    )
                    s = stat_pool.tile([P, 1], f32, tag="s")
                    expt = expt_pool.tile([P, C], f32)
                    # exp(m - E), with row sums accumulated into s
                    nc.scalar.activation(
                        expt[:], e_tiles[ct][:], AF.Exp,
                        bias=m[:], scale=-1.0, accum_out=s[:],
                    )
                    r = stat_pool.tile([P, 1], f32, tag="r")
                    nc.vector.reciprocal(r[:], s[:])
                    rg = stat_pool.tile([P, 1], f32, tag="rg")
                    nc.vector.tensor_mul(rg[:], r[:], gamma_sb[:])
                    attm = attm_pool.tile([P, C], f32)
                    nc.vector.tensor_scalar_mul(attm[:], expt[:], rg[:])
                    attm_tiles.append(attm)

                # ---- phase 3: attT = (gamma*att).T + I ----
                attT_tiles = [attT_pool.tile([P, C], f32r, tag="attT", name="attT_t") for _ in range(CT)]
                for ct in range(CT):
                    for dt in range(CT):
                        tp = ps_pool.tile([P, P], f32, tag="ps", name="tp2")
                        nc.tensor.transpose(
                            tp[:], attm_tiles[ct][:, dt * P : (dt + 1) * P], ident[:]
                        )
                        dst = attT_tiles[dt][:, ct * P : (ct + 1) * P]
                        if ct == dt:
                            nc.vector.tensor_add(dst, tp[:], ident[:])
                        else:
                            nc.vector.tensor_copy(out=dst, in_=tp[:])

                # ---- phase 4: out = (gamma*att + I).T.T @ x ----
                for ct in range(CT):
                    osb = out_pool.tile([P, N], f32)
                    for nch in range(NCH):
                        ops = ps_pool.tile([P, NF], f32, tag="ps")
                        for dt in range(CT):
                            nc.tensor.matmul(
                                ops[:],
                                lhsT=attT_tiles[dt][:, ct * P : (ct + 1) * P],
                                rhs=xnat[:, dt, nch * NF : (nch + 1) * NF],
                                start=(dt == 0),
                                stop=(dt == CT - 1),
                            )
                        eng = nc.scalar if ct % 2 == 0 else nc.vector
                        copy_on(eng, osb[:, nch * NF : (nch + 1) * NF], ops[:])
                    nc.sync.dma_start(
                        out=o_ap[b, ct * P : (ct + 1) * P, :], in_=osb[:]
                    )
    return nc


def _split_excess_waits(nc, max_waits=1):
    """Walrus codegen rejects instructions with more sync-waits than the HW
    instruction struct can hold (self-loading fp32 matmuls hold just one).
    Hoist excess waits onto standalone InstEventSemaphore ops (the same thing
    engine.wait_ge emits) placed immediately before, on the same engine."""
    import concourse.mybir as mybir

    ctr = 0
    for fn in nc.m.functions:
        for blk in fn.blocks:
            out = []
            for inst in blk.instructions:
                si = getattr(inst, "sync_info", None)
                if si is not None and si.on_wait and len(si.on_wait) > max_waits:
                    extra, keep = si.on_wait[:-max_waits], si.on_wait[-max_waits:]
                    for w in extra:
                        ctr += 1
                        ev = mybir.InstEventSemaphore(
                            name=f"WSPLIT-{ctr}", ins=[], outs=[]
                        )
                        ev.engine = inst.engine
                        ev.sync_info = mybir.SyncInfo(on_wait=[w], on_update=[])
                        out.append(ev)
                    inst.sync_info = mybir.SyncInfo(
                        on_wait=keep, on_update=si.on_update
                    )
                out.append(inst)
            blk.instructions[:] = out
    return ctr


def _get_nc():
    if "nc" not in _CACHE:
        nc = _build_bass()
        _split_excess_waits(nc)
        _CACHE["nc"] = nc
    return _CACHE["nc"]


def _get_runner():
    """Build the shard_map-jitted executable once and cache it.

    Mirrors bass2jax.run_bass_via_pjrt's multi-core path, but persists the
    jitted function so repeat kernel() calls skip retracing."""
    if "runner" in _CACHE:
        return _CACHE["runner"]
    import jax
    import numpy as _np
    from jax.sharding import Mesh, PartitionSpec
    from jax.experimental.shard_map import shard_map
    from concourse import bass2jax

    bass2jax.install_neuronx_cc_hook()
    nc = _get_nc()
    out_aval = jax.core.ShapedArray((B_PER, C, N), _np.float32)

    pname = nc.partition_id_tensor.name if nc.partition_id_tensor else None
    in_names = ("x", "gamma", "out") + ((pname,) if pname else ())

    def _body(x_sh, g_sh, out_zero):
        operands = [x_sh, g_sh, out_zero]
        if pname:
            operands.append(bass2jax.partition_id_tensor())
        outs = bass2jax._bass_exec_p.bind(
            *operands,
            out_avals=(out_aval,),
            in_names=in_names,
            out_names=("out",),
            lowering_input_output_aliases=(),
            sim_require_finite=True,
            sim_require_nnan=True,
            nc=nc,
        )
        return outs[0]

    devices = jax.devices()[:N_CORES]
    mesh = Mesh(_np.asarray(devices), ("core",))
    sharded = jax.jit(
        shard_map(
            _body, mesh=mesh,
            in_specs=(PartitionSpec("core"),) * 3,
            out_specs=PartitionSpec("core"),
            check_rep=False,
        ),
        donate_argnums=(2,),
        keep_unused=True,
    )
    _CACHE["runner"] = (sharded, mesh)
    return _CACHE["runner"]


def kernel(x: np.ndarray, gamma: np.ndarray) -> np.ndarray:
    import numpy as _np

    x = np.ascontiguousarray(np.asarray(x), dtype=np.float32)
    gamma = np.ascontiguousarray(np.asarray(gamma), dtype=np.float32)
    sharded, _ = _get_runner()
    g_rep = _np.broadcast_to(gamma.reshape(1), (N_CORES,)).copy()
    out_zero = _np.zeros((B, C, N), _np.float32)
    out = sharded(x, g_rep, out_zero)
    return _np.asarray(out)


# revision 15
# speedup vs baseline: 4140.5603x; 21.7423x over previous
"""CAM (channel attention module) kernel for trn2, batch-parallel over 8 cores.

Per batch item b (x_b: [C=512, N=4096] fp32):
  E = x_b @ x_b.T                  [C, C]   (fp32r matmuls, contraction over N)
  att = softmax(rowmax(E) - E)              == row-softmax of (-E) (shift invariant)
  out = gamma * (att @ x_b) + x_b           (residual folded in as (gamma*att + I) @ x)

Each core handles B/8 = 4 batch items. Inputs arrive full; sharded here.
"""

import numpy as np

B, C, N = 32, 512, 4096
N_CORES = 8
B_PER = B // N_CORES  # 4
P = 128
CT = C // P   # 4 c-tiles
KT = N // P   # 32 k-tiles over N
NF = 512      # matmul2 free-dim chunk (one PSUM bank of fp32)
NCH = N // NF  # 8

_CACHE = {}


def _build_bass():
    import concourse.bass as bass
    import concourse.mybir as mybir
    import concourse.tile as tile
    from concourse import masks

    f32 = mybir.dt.float32
    f32r = mybir.dt.float32r
    AX = mybir.AxisListType
    OP = mybir.AluOpType
    AF = mybir.ActivationFunctionType

    nc = bass.Bass("TRN2", debug=False)
    x_d = nc.dram_tensor("x", [B_PER, C, N], f32, kind="ExternalInput")
    g_d = nc.dram_tensor("gamma", [1], f32, kind="ExternalInput")
    o_d = nc.dram_tensor("out", [B_PER, C, N], f32, kind="ExternalOutput")
    x_ap, g_ap, o_ap = x_d.ap(), g_d.ap(), o_d.ap()

    def copy_on(eng, out, in_):
        if eng is nc.scalar:
            nc.scalar.copy(out=out, in_=in_)
        else:
            nc.vector.tensor_copy(out=out, in_=in_)

    with tile.TileContext(nc) as tc:
        with (
            tc.tile_pool(name="xnat", bufs=2) as xnat_pool,
            tc.tile_pool(name="xt", bufs=4) as xt_pool,
            tc.tile_pool(name="consts", bufs=1) as const_pool,
            tc.tile_pool(name="expt", bufs=2) as expt_pool,
            tc.tile_pool(name="attm", bufs=6) as attm_pool,
            tc.tile_pool(name="attT", bufs=6) as attT_pool,
            tc.tile_pool(name="stats", bufs=16) as stat_pool,
            tc.tile_pool(name="outsb", bufs=2) as out_pool,
            tc.tile_pool(name="ps", bufs=4, space="PSUM") as ps_pool,
            tc.tile_pool(name="eps", bufs=4, space="PSUM") as e_pool,
        ):
            ident = const_pool.tile([P, P], f32)
            masks.make_identity(nc, ident[:])
            ident_r = const_pool.tile([P, P], f32r)
            nc.vector.tensor_copy(out=ident_r[:], in_=ident[:])
            gamma_sb = const_pool.tile([P, 1], f32)
            nc.gpsimd.dma_start(out=gamma_sb[:], in_=g_ap.to_broadcast((P, 1)))

            for b in range(B_PER):
                # ---- load x_b as 4 row-tiles [128, 4096] (one 8MB DMA) ----
                xnat = xnat_pool.tile([P, CT, N], f32r)
                nc.sync.dma_start(
                    out=xnat[:],
                    in_=x_ap[b].rearrange("(j p) n -> p j n", p=P).bitcast(f32r),
                )

                # ---- phase 1: transpose x + E = x @ x.T (accumulate over kn) ----
                e_tiles = [e_pool.tile([P, C], f32, tag="e", name="e_t") for _ in range(CT)]
                for kn in range(KT):
                    xt = xt_pool.tile([P, C], f32r)
                    for j in range(CT):
                        tp = ps_pool.tile([P, P], f32r, tag="ps", name="tp")
                        nc.tensor.transpose(
                            tp[:], xnat[:, j, kn * P : (kn + 1) * P], ident_r[:]
                        )
                        eng = nc.scalar if kn % 2 == 0 else nc.vector
                        copy_on(eng, xt[:, j * P : (j + 1) * P], tp[:])
                    for ct in range(CT):
                        nc.tensor.matmul(
                            e_tiles[ct][:],
                            lhsT=xt[:, ct * P : (ct + 1) * P],
                            rhs=xt[:],
                            start=(kn == 0),
                            stop=(kn == KT - 1),
                        )

                # ---- phase 2: row softmax of (-E), scaled by gamma ----
                attm_tiles = []
                for ct in range(CT):
                    m = stat_pool.tile([P, 1], f32, tag="m")
                    nc.vector.tensor_reduce(
                        m[:], e_tiles[ct][:], axis=AX.X, op=OP.min
                    )
                    s = stat_pool.tile([P, 1], f32, tag="s")
                    expt = expt_pool.tile([P, C], f32)
                    # exp(m - E), with row sums accumulated into s
                    nc.scalar.activation(
                        expt[:], e_tiles[ct][:], AF.Exp,
                        bias=m[:], scale=-1.0, accum_out=s[:],
                    )
                    r = stat_pool.tile([P, 1], f32, tag="r")
                    nc.vector.reciprocal(r[:], s[:])
                    rg = stat_pool.tile([P, 1], f32, tag="rg")
                    nc.vector.tensor_mul(rg[:], r[:], gamma_sb[:])
                    attm = attm_pool.tile([P, C], f32)
                    nc.vector.tensor_scalar_mul(attm[:], expt[:], rg[:])
                    attm_tiles.append(attm)

                # ---- phase 3: attT = (gamma*att).T + I ----
                attT_tiles = [attT_pool.tile([P, C], f32r, tag="attT", name="attT_t") for _ in range(CT)]
                for ct in range(CT):
                    for dt in range(CT):
                        tp = ps_pool.tile([P, P], f32, tag="ps", name="tp2")
                        nc.tensor.transpose(
                            tp[:], attm_tiles[ct][:, dt * P : (dt + 1) * P], ident[:]
                        )
                        dst = attT_tiles[dt][:, ct * P : (ct + 1) * P]
                        if ct == dt:
                            nc.vector.tensor_add(dst, tp[:], ident[:])
                        else:
                            nc.vector.tensor_copy(out=dst, in_=tp[:])

                # ---- phase 4: out = (gamma*att + I).T.T @ x ----
                for ct in range(CT):
                    osb = out_pool.tile([P, N], f32)
                    for nch in range(NCH):
                        ops = ps_pool.tile([P, NF], f32, tag="ps")
                        for dt in range(CT):
                            nc.tensor.matmul(
                                ops[:],
                                lhsT=attT_tiles[dt][:, ct * P : (ct + 1) * P],
                                rhs=xnat[:, dt, nch * NF : (nch + 1) * NF],
                                start=(dt == 0),
                                stop=(dt == CT - 1),
                            )
                        eng = nc.scalar if ct % 2 == 0 else nc.vector
                        copy_on(eng, osb[:, nch * NF : (nch + 1) * NF], ops[:])
                    nc.sync.dma_start(
                        out=o_ap[b, ct * P : (ct + 1) * P, :], in_=osb[:]
                    )
    return nc


def _split_excess_waits(nc, max_waits=1):
    """Walrus codegen rejects instructions with more sync-waits than the HW
    instruction struct can hold (self-loading fp32 matmuls hold just one).
    Hoist excess waits onto standalone InstEventSemaphore ops (the same thing
    engine.wait_ge emits) placed immediately before, on the same engine."""
    import concourse.mybir as mybir

    ctr = 0
    for fn in nc.m.functions:
        for blk in fn.blocks:
            out = []
            for inst in blk.instructions:
                si = getattr(inst, "sync_info", None)
                if si is not None and si.on_wait and len(si.on_wait) > max_waits:
                    extra, keep = si.on_wait[:-max_waits], si.on_wait[-max_waits:]
                    for w in extra:
                        ctr += 1
                        ev = mybir.InstEventSemaphore(
                            name=f"WSPLIT-{ctr}", ins=[], outs=[]
                        )
                        ev.engine = inst.engine
                        ev.sync_info = mybir.SyncInfo(on_wait=[w], on_update=[])
                        out.append(ev)
                    inst.sync_info = mybir.SyncInfo(
                        on_wait=keep, on_update=si.on_update
                    )
                out.append(inst)
            blk.instructions[:] = out
    return ctr


def _get_nc():
    if "nc" not in _CACHE:
        nc = _build_bass()
        _split_excess_waits(nc)
        _CACHE["nc"] = nc
    return _CACHE["nc"]


def _get_runner(n_chain: int = 1):
    """Build the shard_map-jitted executable once and cache it.

    Mirrors bass2jax.run_bass_via_pjrt's multi-core path, but persists the
    jitted function so repeat kernel() calls skip retracing. n_chain > 1
    runs the NEFF that many times back-to-back inside one dispatch (each
    run needs its own donated zero output buffer) - used for timing."""
    key = ("runner", n_chain)
    if key in _CACHE:
        return _CACHE[key]
    import jax
    import numpy as _np
    from jax.sharding import Mesh, PartitionSpec
    from jax.experimental.shard_map import shard_map
    from concourse import bass2jax

    bass2jax.install_neuronx_cc_hook()
    nc = _get_nc()
    out_aval = jax.core.ShapedArray((B_PER, C, N), _np.float32)

    pname = nc.partition_id_tensor.name if nc.partition_id_tensor else None
    in_names = ("x", "gamma", "out") + ((pname,) if pname else ())

    def _body(x_sh, g_sh, out_zero):
        operands = [x_sh, g_sh, out_zero]
        if pname:
            operands.append(bass2jax.partition_id_tensor())
        outs = bass2jax._bass_exec_p.bind(
            *operands,
            out_avals=(out_aval,),
            in_names=in_names,
            out_names=("out",),
            lowering_input_output_aliases=(),
            sim_require_finite=True,
            sim_require_nnan=True,
            nc=nc,
        )
        return outs[0]

    def _chain(x_sh, g_sh, *zeros):
        o = None
        for z in zeros:
            o = _body(x_sh, g_sh, z)
        return o

    devices = jax.devices()[:N_CORES]
    mesh = Mesh(_np.asarray(devices), ("core",))
    sharded = jax.jit(
        shard_map(
            _chain, mesh=mesh,
            in_specs=(PartitionSpec("core"),) * (2 + n_chain),
            out_specs=PartitionSpec("core"),
            check_rep=False,
        ),
        donate_argnums=tuple(range(2, 2 + n_chain)),
        keep_unused=True,
    )
    _CACHE[key] = (sharded, mesh)
    return _CACHE[key]


def kernel(x: np.ndarray, gamma: np.ndarray) -> np.ndarray:
    import numpy as _np

    x = np.ascontiguousarray(np.asarray(x), dtype=np.float32)
    gamma = np.ascontiguousarray(np.asarray(gamma), dtype=np.float32)
    sharded, _ = _get_runner()
    g_rep = _np.broadcast_to(gamma.reshape(1), (N_CORES,)).copy()
    out_zero = _np.zeros((B, C, N), _np.float32)
    out = sharded(x, g_rep, out_zero)
    return _np.asarray(out)
